# revision 25
# baseline (speedup 1.0000x reference)
"""Trainium2 Bass kernel for nn_CaptionModel (GRU caption decoder).

Model: h0 = feat; x0 = embed[<SOS>]; 200 GRU steps where the output hidden
state is fed back as the next input (x_t = h_t for t >= 1), then a linear
projection of every hidden state to vocab logits, output [B, V, T].

Since x_t == h_t for t >= 1 the two GRU matmuls fuse into one 2048-wide
gate matmul G = h @ Wc.T + bc with Wc = [w_ih_r+w_hh_r; -(w_ih_z+w_hh_z);
w_hh_n; w_ih_n] (z negated so sig(psum) = 1-z), gates r = sig(G0),
z1m = sig(G1) = 1-z, n = tanh(G3 + r*G2), h' = (h - z1m*h) + z1m*n.

KEY STRUCTURE: the GRU recurrence is latency-bound -- the serial loop
[e-operand gate mms -> sigmoid -> a=r*g_hn -> b=a+g_in -> tanh ->
e=z1m*n -> next mms] crosses engines ~6 times at ~200-300ns per handoff
(sem delay + producer pipeline drain), so one recurrence cycles at
~2.5-3us regardless of batch size.  The per-core batch of 32 is split
into TWO independent 16-sample recurrences interleaved on the same
engines (every engine is <60% busy in a single recurrence); each
sub-recurrence then cycles slightly faster (~2.5us, smaller ops) and
they overlap, landing at ~2.6us/step with the PE ~79% busy.

Per half: gate PSUM is 2 single banks (RZ: r|z, NI: hn|in) -- PSUM
dependency tracking is bank-granular, so each bank is fully written
(bias mms first, start=True zeroes the whole 2KB bank) then read.
Burst order per step: bias + u-operand mms (u = z*h is ready early via
the Pool path), then e-operand mms rz-first so the sigmoid fires right
after the 32 e_rz mms instead of after the whole burst.  The sigmoid is
split r-half / z-half (both read the completed RZ bank) so r feeds the
critical a->b->tanh chain one op earlier; z1m feeds the off-loop u-path
(t1 = z1m*h, u = h - t1 on Pool) and the e-mul.  e16 is written in
c-halves so the next burst's e_rz(c01) mms start early; the hist join
(h' = u+e) rides the DVE queue after e16, off the critical loop.

All fp16 rounding sites (t1, u16, e16, n16, hist) are placed exactly as
in the validated single-recurrence kernel: the 200-step feedback loop
amplifies any change in rounding placement, so the math is kept bitwise
identical (rel err 1.92e-2 vs the 2e-2 gate).

Sharding: pure data parallelism, batch 256 -> 32 per core on 8 cores,
weights replicated.
"""

import os
from contextlib import ExitStack

import numpy as np

import concourse.bass as bass
import concourse.tile as tile
from concourse import bacc, mybir
from concourse.bass_utils import run_bass_kernel_spmd

B, H, VOCAB = 256, 512, 100
STEPS = int(os.environ.get("KERNEL_STEPS", "200"))
NCORES = 8
BD = B // NCORES  # 32
HB = BD // 2      # 16 per interleaved half-recurrence
F16 = mybir.dt.float16
F32 = mybir.dt.float32
SIG = mybir.ActivationFunctionType.Sigmoid
TANH = mybir.ActivationFunctionType.Tanh

# gate order in the stationary weight blocks: r z hn in
GATES = ("r", "z", "hn", "in")
GI = {g: i for i, g in enumerate(GATES)}
PROJ_EVERY = 8


def _blk(g, q, c):
    return ((GI[g] * 4 + q) * 4 + c) * 128


def _colof(g, q):
    # within-bank columns: r/hn at q*HB (cols 0:64), z/in at 64+q*HB
    return (64 if g in ("z", "in") else 0) + q * HB


def _build(steps: int):
    nc = bacc.Bacc("TRN2", target_bir_lowering=False, debug=False,
                   num_devices=NCORES)
    T1 = steps + 1

    wst_d = nc.dram_tensor("wst", [128, 64 * 128], F16, kind="ExternalInput").ap()
    wst0_d = nc.dram_tensor("wst0", [128, 32 * 128], F16, kind="ExternalInput").ap()
    h0_d = nc.dram_tensor("h0", [128, 128], F16, kind="ExternalInput").ap()
    bt_d = nc.dram_tensor("bt", [1, 2048], F16, kind="ExternalInput").ap()
    bt0_d = nc.dram_tensor("bt0", [1, 2048], F16, kind="ExternalInput").ap()
    ones_d = nc.dram_tensor("ones", [1, BD], F16, kind="ExternalInput").ap()
    pjt_d = nc.dram_tensor("pjt", [128, 4 * VOCAB], F16, kind="ExternalInput").ap()
    pjb_d = nc.dram_tensor("pjb", [VOCAB, 1], F32, kind="ExternalInput").ap()
    out_d = nc.dram_tensor("out", [BD, VOCAB, steps], F32,
                           kind="ExternalOutput").ap()

    with tile.TileContext(nc) as tc, ExitStack() as ctx:
        sg = ctx.enter_context(tc.tile_pool(name="sg", bufs=1))
        wk = ctx.enter_context(tc.tile_pool(name="wk", bufs=3))

        wst = sg.tile([128, 64 * 128], F16)
        nc.sync.dma_start(out=wst, in_=wst_d)
        wst0 = sg.tile([128, 32 * 128], F16)
        nc.sync.dma_start(out=wst0, in_=wst0_d)
        bt = sg.tile([1, 2048], F16)
        nc.sync.dma_start(out=bt, in_=bt_d)
        bt0 = sg.tile([1, 2048], F16)
        nc.sync.dma_start(out=bt0, in_=bt0_d)
        ones = sg.tile([1, BD], F16)
        nc.sync.dma_start(out=ones, in_=ones_d)
        pjt = sg.tile([128, 4 * VOCAB], F16)
        nc.sync.dma_start(out=pjt, in_=pjt_d)
        pjb = sg.tile([VOCAB, 1], F32)
        nc.sync.dma_start(out=pjb, in_=pjb_d)
        hist = sg.tile([128, 4, T1, BD], F16, name="hist")
        nc.sync.dma_start(out=hist[:, :, 0, :],
                          in_=h0_d.rearrange("p (q b) -> p q b", q=4))
        stage = sg.tile([VOCAB, BD * steps], F32, name="stage")

        with tc.tile_pool(name="gps", bufs=1, space="PSUM") as gpool, \
             tc.tile_pool(name="pps", bufs=2, space="PSUM") as ppool:
            # two single banks per half: RZ (r|z) and NI (hn|in).  Bank-
            # granular deps + whole-bank zeroing mean each accumulation
            # group owns a full 2KB bank; 4 gate banks + 2 proj banks = 6.
            # One combined sigmoid over [r|z1m] feeds both the e-chain (r)
            # and the u-path (z1m) in a single Act op.
            RZ = [gpool.tile([128, 512], F32, tag=f"RZ{i}", name=f"RZ{i}")
                  for i in range(2)]
            NI = [gpool.tile([128, 512], F32, tag=f"NI{i}", name=f"NI{i}")
                  for i in range(2)]

            def bank_of(X, g):
                return RZ[X] if g in ("r", "z") else NI[X]

            def bias_u_mms(X, gates, bias_src, w_src, rhs4, wgates=None):
                # first fill phase of each bank: bias mms (first carries
                # start=True, zeroing the bank) then the early-operand
                # weight mms.  wgates limits which gates get weight mms.
                hb0 = X * HB
                if wgates is None:
                    wgates = gates
                firsts = set()
                for g in gates:
                    bank = bank_of(X, g)
                    for q in range(4):
                        col = _colof(g, q)
                        first = id(bank) not in firsts
                        firsts.add(id(bank))
                        nc.tensor.matmul(
                            bank[:, col:col + HB],
                            bias_src[:, GI[g] * 512 + q * 128:
                                     GI[g] * 512 + (q + 1) * 128],
                            ones[:, 0:HB], start=first, stop=False,
                            skip_group_check=True)
                for g in wgates:
                    bank = bank_of(X, g)
                    for q in range(4):
                        for c in range(4):
                            wt = wst0 if (w_src is wst0 and g in ("r", "z")) \
                                else wst
                            nc.tensor.matmul(
                                bank[:, _colof(g, q):_colof(g, q) + HB],
                                wt[:, _blk(g, q, c):_blk(g, q, c) + 128],
                                rhs4[c],
                                start=False,
                                stop=(w_src is wst0 and q == 3 and c == 3),
                                skip_group_check=True)

            def e_mms(X, gates, rhs4, stop_gates, cs=(0, 1, 2, 3)):
                # second fill phase: the late-operand weight mms; gates in
                # stop_gates close their bank's group on their last mm
                for g in gates:
                    bank = bank_of(X, g)
                    for q in range(4):
                        for c in cs:
                            stop = (g in stop_gates and q == 3 and c == cs[-1])
                            nc.tensor.matmul(
                                bank[:, _colof(g, q):_colof(g, q) + HB],
                                wst[:, _blk(g, q, c):_blk(g, q, c) + 128],
                                rhs4[c],
                                start=False, stop=stop,
                                skip_group_check=True)

            def proj_rows(X, r0, nrows):
                hb0 = X * HB
                Pfull = ppool.tile([VOCAB, 512], F32, tag="P", name="Pfull")
                P = Pfull[:, 0:HB * nrows]
                for c in range(4):
                    rhs = hist[:, c, r0:r0 + nrows, hb0:hb0 + HB].rearrange(
                        "p t b -> p b t")
                    nc.tensor.matmul(P, pjt[:, c * VOCAB:(c + 1) * VOCAB], rhs,
                                     start=(c == 0), stop=(c == 3))
                st_sl = stage.rearrange("p (b t) -> p b t", b=BD)[
                    :, hb0:hb0 + HB, r0 - 1:r0 - 1 + nrows]
                nc.vector.tensor_scalar_add(
                    st_sl, P.rearrange("p (b t) -> p b t", b=HB), pjb)

            # ---- step 0: bias0 + w_hh matmuls on h0 (in-region bias-only)
            for X in (0, 1):
                h0c = [hist[:, c, 0, X * HB:X * HB + HB] for c in range(4)]
                bias_u_mms(X, ("r", "z", "hn", "in"), bt0, wst0, h0c,
                           wgates=("r", "z", "hn"))

            next_proj = [1, 1]
            dma_done = 0
            DMA_CHUNK = 50
            for t in range(steps):
                for X in (0, 1):
                    hb0 = X * HB
                    rz_s = wk.tile([128, 128], F32, tag=f"rz{X}")
                    a_s = wk.tile([128, 64], F32, tag=f"a{X}")
                    b_s = wk.tile([128, 64], F32, tag=f"b{X}")
                    n16 = wk.tile([128, 64], F16, tag=f"n{X}")
                    # t1 in f16 keeps the u-path math bitwise identical to the
                    # validated single-recurrence kernel (final max-err of this
                    # 200-step feedback loop is sensitive to rounding placement)
                    t1 = wk.tile([128, 4, HB], F16, tag=f"t1{X}")
                    u16 = wk.tile([128, 4, HB], F16, tag=f"u{X}")
                    e16 = wk.tile([128, 4, HB], F16, tag=f"e{X}")
                    h_t = hist[:, :, t, hb0:hb0 + HB]

                    # chain for step t (banks were filled by iteration t-1).
                    # critical loop: e_rz mms -> sig_r -> a -> b -> tanh ->
                    # e16 -> (next burst's e mms).  The u-path (sig_z1m ->
                    # t1 -> u on Pool) runs in parallel off-loop; splitting
                    # the sigmoid lets the r half feed the chain 54ns
                    # earlier (both halves read the same completed bank)
                    nc.scalar.activation(rz_s[:, 0:64], RZ[X][:, 0:64], SIG)
                    nc.scalar.activation(rz_s[:, 64:128], RZ[X][:, 64:128],
                                         SIG)
                    z1m4 = rz_s[:, 64:128].rearrange("p (q b) -> p q b", q=4)
                    # u path on Pool: t1 = (1-z)*h in f32, u = h - t1
                    nc.gpsimd.tensor_mul(t1, z1m4, h_t)
                    nc.gpsimd.tensor_sub(u16, h_t, t1)
                    # e path on DVE/Act; e16 in c-halves so the next
                    # burst's e_rz(c01) mms start early; the hist join
                    # rides the DVE queue after e16 (off the critical loop)
                    nc.vector.tensor_mul(a_s, rz_s[:, 0:64], NI[X][:, 0:64])
                    nc.vector.tensor_add(b_s, a_s, NI[X][:, 64:128])
                    nc.scalar.activation(n16, b_s, TANH)
                    n4 = n16.rearrange("p (q b) -> p q b", q=4)
                    nc.vector.tensor_mul(e16[:, 0:2, :], z1m4[:, 0:2, :],
                                         n4[:, 0:2, :])
                    nc.vector.tensor_mul(e16[:, 2:4, :], z1m4[:, 2:4, :],
                                         n4[:, 2:4, :])
                    nc.vector.tensor_add(hist[:, :, t + 1, hb0:hb0 + HB],
                                         u16, e16)

                    # burst for G(t+1): bias + u-operand mms first (u is
                    # ready early via the Pool path), then the e-operand
                    # mms rz-first (c01 then c23) so sig(t+1) fires after
                    # only the 32 e_rz mms
                    if t + 1 < steps:
                        u4 = [u16[:, c, :] for c in range(4)]
                        e4 = [e16[:, c, :] for c in range(4)]
                        bias_u_mms(X, ("r", "z", "hn", "in"), bt, wst, u4)
                        e_mms(X, ("r", "z"), e4, (), cs=(0, 1))
                        e_mms(X, ("r", "z"), e4, ("z",), cs=(2, 3))
                        e_mms(X, ("hn", "in"), e4, ("in",))

                    if next_proj[X] + PROJ_EVERY <= t:
                        proj_rows(X, next_proj[X], PROJ_EVERY)
                        next_proj[X] += PROJ_EVERY

                # stream finalized output chunks once BOTH halves are done
                while dma_done + DMA_CHUNK < min(next_proj):
                    lo = dma_done
                    nc.sync.dma_start(
                        out=out_d[:, :, lo:lo + DMA_CHUNK].rearrange(
                            "b v t -> v b t"),
                        in_=stage.rearrange("p (b t) -> p b t", b=BD)[
                            :, :, lo:lo + DMA_CHUNK])
                    dma_done += DMA_CHUNK

            for X in (0, 1):
                while next_proj[X] <= steps:
                    nrows = min(PROJ_EVERY, steps + 1 - next_proj[X])
                    proj_rows(X, next_proj[X], nrows)
                    next_proj[X] += nrows

        if dma_done < steps:
            nc.sync.dma_start(
                out=out_d[:, :, dma_done:].rearrange("b v t -> v b t"),
                in_=stage.rearrange("p (b t) -> p b t", b=BD)[:, :, dma_done:])
    nc.compile()
    return nc


_CACHE = {}


def _get_nc(steps: int):
    if steps not in _CACHE:
        _CACHE[steps] = _build(steps)
    return _CACHE[steps]


def _prep_inputs(feat, embed_table, w_ih, w_hh, b_ih, b_hh, proj_w, proj_b):
    f32 = np.float32
    f16 = np.float16
    w_ih = np.asarray(w_ih, f32)
    w_hh = np.asarray(w_hh, f32)
    b_ih = np.asarray(b_ih, f32)
    b_hh = np.asarray(b_hh, f32)
    # fused gate weights, gate-major order r, z, hn, in
    # z gate negated: sigmoid(z psum) then directly equals 1 - z
    Wc = np.concatenate([w_ih[:H] + w_hh[:H],
                         -(w_ih[H:2 * H] + w_hh[H:2 * H]),
                         w_hh[2 * H:],
                         w_ih[2 * H:]], 0)          # [4H, H]
    bc = np.concatenate([b_ih[:H] + b_hh[:H],
                         -(b_ih[H:2 * H] + b_hh[H:2 * H]),
                         b_hh[2 * H:],
                         b_ih[2 * H:]], 0)          # [4H]

    x0 = np.asarray(embed_table, f32)[0]
    gi0 = w_ih @ x0 + b_ih                          # [3H]
    bc0 = np.concatenate([gi0[:H] + b_hh[:H],
                          -(gi0[H:2 * H] + b_hh[H:2 * H]),
                          b_hh[2 * H:],
                          gi0[2 * H:]], 0)          # [4H]
    W0 = np.concatenate([w_hh[:H], -w_hh[H:2 * H]], 0)  # [2H, H] r,z step-0

    # stationary blocks: wst[kp, ((g*4+q)*4+c)*128 + m] = Wc[g*512+q*128+m,
    #                                                        c*128+kp]
    wst = np.empty((128, 64 * 128), f32)
    for g in range(4):
        for q in range(4):
            for c in range(4):
                blk = ((g * 4 + q) * 4 + c) * 128
                wst[:, blk:blk + 128] = Wc[g * 512 + q * 128:
                                           g * 512 + (q + 1) * 128,
                                           c * 128:(c + 1) * 128].T
    wst0 = np.empty((128, 32 * 128), f32)
    for g in range(2):
        for q in range(4):
            for c in range(4):
                blk = ((g * 4 + q) * 4 + c) * 128
                wst0[:, blk:blk + 128] = W0[g * 512 + q * 128:
                                            g * 512 + (q + 1) * 128,
                                            c * 128:(c + 1) * 128].T

    proj_w = np.asarray(proj_w, f32)                # [V, H]
    pjt = np.empty((128, 4 * VOCAB), f32)
    for c in range(4):
        pjt[:, c * VOCAB:(c + 1) * VOCAB] = proj_w[:, c * 128:(c + 1) * 128].T

    feat = np.asarray(feat, f32)
    common = {
        "wst": wst.astype(f16),
        "wst0": wst0.astype(f16),
        "bt": bc.reshape(1, 2048).astype(f16),
        "bt0": bc0.reshape(1, 2048).astype(f16),
        "ones": np.ones((1, BD), f16),
        "pjt": pjt.astype(f16),
        "pjb": np.asarray(proj_b, f32).reshape(VOCAB, 1),
    }
    maps = []
    for i in range(NCORES):
        fs = feat[i * BD:(i + 1) * BD]              # [BD, H]
        h0g = np.ascontiguousarray(
            fs.T.reshape(4, 128, BD).transpose(1, 0, 2).reshape(128, 128))
        maps.append(dict(common, h0=h0g.astype(f16)))
    return maps


def kernel(feat, embed_table, w_ih, w_hh, b_ih, b_hh, proj_w, proj_b,
           _trace=False):
    nc = _get_nc(STEPS)
    in_maps = _prep_inputs(feat, embed_table, w_ih, w_hh, b_ih, b_hh,
                           proj_w, proj_b)
    res = run_bass_kernel_spmd(nc, in_maps, list(range(NCORES)), trace=_trace)
    out = np.concatenate([res.results[i]["out"] for i in range(NCORES)], 0)
    if _trace:
        kernel.last_exec_time_ns = res.exec_time_ns
        kernel.last_results = res
    return out


# revision 32
# speedup vs baseline: 1.0000x; 1.0000x over previous
"""Trainium2 Bass kernel for nn_CaptionModel (GRU caption decoder).

Model: h0 = feat; x0 = embed[<SOS>]; 200 GRU steps where the output hidden
state is fed back as the next input (x_t = h_t for t >= 1), then a linear
projection of every hidden state to vocab logits, output [B, V, T].

Since x_t == h_t for t >= 1 the two GRU matmuls fuse into one 2048-wide
gate matmul G = h @ Wc.T + bc with Wc = [w_ih_r+w_hh_r; -(w_ih_z+w_hh_z);
w_hh_n; w_ih_n] (z negated so sig(psum) = 1-z), gates r = sig(G0),
z1m = sig(G1) = 1-z, n = tanh(G3 + r*G2), h' = (h - z1m*h) + z1m*n.

KEY STRUCTURE: the GRU recurrence is latency-bound -- the serial loop
[e-operand gate mms -> sigmoid -> a=r*g_hn -> b=a+g_in -> tanh ->
e=z1m*n -> next mms] crosses engines ~6 times at ~200-300ns per handoff
(sem delay + producer pipeline drain), so one recurrence cycles at
~2.5-3us regardless of batch size.  The per-core batch of 32 is split
into TWO independent 16-sample recurrences interleaved on the same
engines (every engine is <60% busy in a single recurrence); each
sub-recurrence then cycles slightly faster (~2.5us, smaller ops) and
they overlap, landing at ~2.6us/step with the PE ~79% busy.

Per half: gate PSUM is 2 single banks (RZ: r|z, NI: hn|in) -- PSUM
dependency tracking is bank-granular, so each bank is fully written
(bias mms first, start=True zeroes the whole 2KB bank) then read.
Burst order per step: bias + u-operand mms (u = z*h is ready early via
the Pool path), then e-operand mms rz-first so the sigmoid fires right
after the 32 e_rz mms instead of after the whole burst.  The sigmoid is
split r-half / z-half (both read the completed RZ bank) so r feeds the
critical a->b->tanh chain one op earlier; z1m feeds the off-loop u-path
(t1 = z1m*h, u = h - t1 on Pool) and the e-mul.  e16 is written in
c-halves so the next burst's e_rz(c01) mms start early; the hist join
(h' = u+e) rides the DVE queue after e16, off the critical loop.

All fp16 rounding sites (t1, u16, e16, n16, hist) are placed exactly as
in the validated single-recurrence kernel: the 200-step feedback loop
amplifies any change in rounding placement, so the math is kept bitwise
identical (rel err 1.92e-2 vs the 2e-2 gate).

Sharding: pure data parallelism, batch 256 -> 32 per core on 8 cores,
weights replicated.
"""

import os
from contextlib import ExitStack

import numpy as np

import concourse.bass as bass
import concourse.tile as tile
from concourse import bacc, mybir
from concourse.bass_utils import run_bass_kernel_spmd

B, H, VOCAB = 256, 512, 100
STEPS = int(os.environ.get("KERNEL_STEPS", "200"))
NCORES = 8
BD = B // NCORES  # 32
HB = BD // 2      # 16 per interleaved half-recurrence
F16 = mybir.dt.float16
F32 = mybir.dt.float32
SIG = mybir.ActivationFunctionType.Sigmoid
TANH = mybir.ActivationFunctionType.Tanh

# gate order in the stationary weight blocks: r z hn in
GATES = ("r", "z", "hn", "in")
GI = {g: i for i, g in enumerate(GATES)}
PROJ_EVERY = 8


def _blk(g, q, c):
    return ((GI[g] * 4 + q) * 4 + c) * 128


def _colof(g, q):
    # within-bank columns: r/hn at q*HB (cols 0:64), z/in at 64+q*HB
    return (64 if g in ("z", "in") else 0) + q * HB


def _build(steps: int):
    nc = bacc.Bacc("TRN2", target_bir_lowering=False, debug=False,
                   num_devices=NCORES)
    T1 = steps + 1

    wst_d = nc.dram_tensor("wst", [128, 64 * 128], F16, kind="ExternalInput").ap()
    wst0_d = nc.dram_tensor("wst0", [128, 32 * 128], F16, kind="ExternalInput").ap()
    h0_d = nc.dram_tensor("h0", [128, 128], F16, kind="ExternalInput").ap()
    bt_d = nc.dram_tensor("bt", [1, 2048], F16, kind="ExternalInput").ap()
    bt0_d = nc.dram_tensor("bt0", [1, 2048], F16, kind="ExternalInput").ap()
    ones_d = nc.dram_tensor("ones", [1, BD], F16, kind="ExternalInput").ap()
    pjt_d = nc.dram_tensor("pjt", [128, 4 * VOCAB], F16, kind="ExternalInput").ap()
    pjb_d = nc.dram_tensor("pjb", [VOCAB, 1], F32, kind="ExternalInput").ap()
    out_d = nc.dram_tensor("out", [BD, VOCAB, steps], F32,
                           kind="ExternalOutput").ap()

    with tile.TileContext(nc) as tc, ExitStack() as ctx:
        sg = ctx.enter_context(tc.tile_pool(name="sg", bufs=1))
        wk = ctx.enter_context(tc.tile_pool(name="wk", bufs=4))

        wst = sg.tile([128, 64 * 128], F16)
        nc.sync.dma_start(out=wst, in_=wst_d)
        wst0 = sg.tile([128, 32 * 128], F16)
        nc.sync.dma_start(out=wst0, in_=wst0_d)
        bt = sg.tile([1, 2048], F16)
        nc.sync.dma_start(out=bt, in_=bt_d)
        bt0 = sg.tile([1, 2048], F16)
        nc.sync.dma_start(out=bt0, in_=bt0_d)
        ones = sg.tile([1, BD], F16)
        nc.sync.dma_start(out=ones, in_=ones_d)
        pjt = sg.tile([128, 4 * VOCAB], F16)
        nc.sync.dma_start(out=pjt, in_=pjt_d)
        pjb = sg.tile([VOCAB, 1], F32)
        nc.sync.dma_start(out=pjb, in_=pjb_d)
        hist = sg.tile([128, 4, T1, BD], F16, name="hist")
        nc.sync.dma_start(out=hist[:, :, 0, :],
                          in_=h0_d.rearrange("p (q b) -> p q b", q=4))
        stage = sg.tile([VOCAB, BD * steps], F32, name="stage")

        with tc.tile_pool(name="gps", bufs=1, space="PSUM") as gpool, \
             tc.tile_pool(name="pps", bufs=2, space="PSUM") as ppool:
            # two single banks per half: RZ (r|z) and NI (hn|in).  Bank-
            # granular deps + whole-bank zeroing mean each accumulation
            # group owns a full 2KB bank; 4 gate banks + 2 proj banks = 6.
            # One combined sigmoid over [r|z1m] feeds both the e-chain (r)
            # and the u-path (z1m) in a single Act op.
            RZ = [gpool.tile([128, 512], F32, tag=f"RZ{i}", name=f"RZ{i}")
                  for i in range(2)]
            NI = [gpool.tile([128, 512], F32, tag=f"NI{i}", name=f"NI{i}")
                  for i in range(2)]

            def bank_of(X, g):
                return RZ[X] if g in ("r", "z") else NI[X]

            def bias_u_mms(X, gates, bias_src, w_src, rhs4, wgates=None):
                # first fill phase of each bank: bias mms (first carries
                # start=True, zeroing the bank) then the early-operand
                # weight mms.  wgates limits which gates get weight mms.
                hb0 = X * HB
                if wgates is None:
                    wgates = gates
                firsts = set()
                for g in gates:
                    bank = bank_of(X, g)
                    for q in range(4):
                        col = _colof(g, q)
                        first = id(bank) not in firsts
                        firsts.add(id(bank))
                        nc.tensor.matmul(
                            bank[:, col:col + HB],
                            bias_src[:, GI[g] * 512 + q * 128:
                                     GI[g] * 512 + (q + 1) * 128],
                            ones[:, 0:HB], start=first, stop=False,
                            skip_group_check=True)
                for g in wgates:
                    bank = bank_of(X, g)
                    for q in range(4):
                        for c in range(4):
                            wt = wst0 if (w_src is wst0 and g in ("r", "z")) \
                                else wst
                            nc.tensor.matmul(
                                bank[:, _colof(g, q):_colof(g, q) + HB],
                                wt[:, _blk(g, q, c):_blk(g, q, c) + 128],
                                rhs4[c],
                                start=False,
                                stop=(w_src is wst0 and q == 3 and c == 3),
                                skip_group_check=True)

            def e_mms(X, gates, rhs4, stop_gates, cs=(0, 1, 2, 3)):
                # second fill phase: the late-operand weight mms; gates in
                # stop_gates close their bank's group on their last mm
                for g in gates:
                    bank = bank_of(X, g)
                    for q in range(4):
                        for c in cs:
                            stop = (g in stop_gates and q == 3 and c == cs[-1])
                            nc.tensor.matmul(
                                bank[:, _colof(g, q):_colof(g, q) + HB],
                                wst[:, _blk(g, q, c):_blk(g, q, c) + 128],
                                rhs4[c],
                                start=False, stop=stop,
                                skip_group_check=True)

            def proj_rows(X, r0, nrows):
                hb0 = X * HB
                Pfull = ppool.tile([VOCAB, 512], F32, tag="P", name="Pfull")
                P = Pfull[:, 0:HB * nrows]
                for c in range(4):
                    rhs = hist[:, c, r0:r0 + nrows, hb0:hb0 + HB].rearrange(
                        "p t b -> p b t")
                    nc.tensor.matmul(P, pjt[:, c * VOCAB:(c + 1) * VOCAB], rhs,
                                     start=(c == 0), stop=(c == 3))
                st_sl = stage.rearrange("p (b t) -> p b t", b=BD)[
                    :, hb0:hb0 + HB, r0 - 1:r0 - 1 + nrows]
                nc.vector.tensor_scalar_add(
                    st_sl, P.rearrange("p (b t) -> p b t", b=HB), pjb)

            # ---- step 0: bias0 + w_hh matmuls on h0 (in-region bias-only)
            for X in (0, 1):
                h0c = [hist[:, c, 0, X * HB:X * HB + HB] for c in range(4)]
                bias_u_mms(X, ("r", "z", "hn", "in"), bt0, wst0, h0c,
                           wgates=("r", "z", "hn"))

            next_proj = [1, 1]
            dma_done = 0
            DMA_CHUNK = 50
            for t in range(steps):
                for X in (0, 1):
                    hb0 = X * HB
                    rz_s = wk.tile([128, 128], F32, tag=f"rz{X}")
                    a_s = wk.tile([128, 64], F32, tag=f"a{X}")
                    b_s = wk.tile([128, 64], F32, tag=f"b{X}")
                    n16 = wk.tile([128, 64], F16, tag=f"n{X}")
                    # t1 in f16 keeps the u-path math bitwise identical to the
                    # validated single-recurrence kernel (final max-err of this
                    # 200-step feedback loop is sensitive to rounding placement)
                    t1 = wk.tile([128, 4, HB], F16, tag=f"t1{X}")
                    u16 = wk.tile([128, 4, HB], F16, tag=f"u{X}")
                    e16 = wk.tile([128, 4, HB], F16, tag=f"e{X}")
                    h_t = hist[:, :, t, hb0:hb0 + HB]

                    # chain for step t (banks were filled by iteration t-1).
                    # critical loop: e_rz mms -> sig_r -> a -> b -> tanh ->
                    # e16 -> (next burst's e mms).  The u-path (sig_z1m ->
                    # t1 -> u on Pool) runs in parallel off-loop; splitting
                    # the sigmoid lets the r half feed the chain 54ns
                    # earlier (both halves read the same completed bank)
                    nc.scalar.activation(rz_s[:, 0:64], RZ[X][:, 0:64], SIG)
                    nc.scalar.activation(rz_s[:, 64:128], RZ[X][:, 64:128],
                                         SIG)
                    z1m4 = rz_s[:, 64:128].rearrange("p (q b) -> p q b", q=4)
                    # u path on Pool: t1 = (1-z)*h in f32, u = h - t1
                    nc.gpsimd.tensor_mul(t1, z1m4, h_t)
                    nc.gpsimd.tensor_sub(u16, h_t, t1)
                    # e path on DVE/Act; e16 in c-halves so the next
                    # burst's e_rz(c01) mms start early; the hist join
                    # rides the DVE queue after e16 (off the critical loop)
                    nc.vector.tensor_mul(a_s, rz_s[:, 0:64], NI[X][:, 0:64])
                    nc.vector.tensor_add(b_s, a_s, NI[X][:, 64:128])
                    nc.scalar.activation(n16, b_s, TANH)
                    n4 = n16.rearrange("p (q b) -> p q b", q=4)
                    nc.vector.tensor_mul(e16[:, 0:2, :], z1m4[:, 0:2, :],
                                         n4[:, 0:2, :])
                    nc.vector.tensor_mul(e16[:, 2:4, :], z1m4[:, 2:4, :],
                                         n4[:, 2:4, :])
                    # hist join h(t+1) = u+e on DVE, off the critical loop
                    nc.vector.tensor_add(hist[:, :, t + 1, hb0:hb0 + HB],
                                         u16, e16)

                    # burst for G(t+1): bias + u-operand mms first (u is
                    # ready early via the Pool path), then the e-operand
                    # mms rz-first (c01 then c23) so sig(t+1) fires after
                    # only the 32 e_rz mms
                    if t + 1 < steps:
                        u4 = [u16[:, c, :] for c in range(4)]
                        e4 = [e16[:, c, :] for c in range(4)]
                        bias_u_mms(X, ("r", "z", "hn", "in"), bt, wst, u4)
                        e_mms(X, ("r", "z"), e4, (), cs=(0, 1))
                        e_mms(X, ("r", "z"), e4, ("z",), cs=(2, 3))
                        e_mms(X, ("hn", "in"), e4, ("in",))

                    if next_proj[X] + PROJ_EVERY <= t:
                        proj_rows(X, next_proj[X], PROJ_EVERY)
                        next_proj[X] += PROJ_EVERY

                # stream finalized output chunks once BOTH halves are done
                while dma_done + DMA_CHUNK < min(next_proj):
                    lo = dma_done
                    nc.sync.dma_start(
                        out=out_d[:, :, lo:lo + DMA_CHUNK].rearrange(
                            "b v t -> v b t"),
                        in_=stage.rearrange("p (b t) -> p b t", b=BD)[
                            :, :, lo:lo + DMA_CHUNK])
                    dma_done += DMA_CHUNK

            for X in (0, 1):
                while next_proj[X] <= steps:
                    nrows = min(PROJ_EVERY, steps + 1 - next_proj[X])
                    proj_rows(X, next_proj[X], nrows)
                    next_proj[X] += nrows

        if dma_done < steps:
            nc.sync.dma_start(
                out=out_d[:, :, dma_done:].rearrange("b v t -> v b t"),
                in_=stage.rearrange("p (b t) -> p b t", b=BD)[:, :, dma_done:])
    nc.compile()
    return nc


_CACHE = {}


def _get_nc(steps: int):
    if steps not in _CACHE:
        _CACHE[steps] = _build(steps)
    return _CACHE[steps]


def _prep_inputs(feat, embed_table, w_ih, w_hh, b_ih, b_hh, proj_w, proj_b):
    f32 = np.float32
    f16 = np.float16
    w_ih = np.asarray(w_ih, f32)
    w_hh = np.asarray(w_hh, f32)
    b_ih = np.asarray(b_ih, f32)
    b_hh = np.asarray(b_hh, f32)
    # fused gate weights, gate-major order r, z, hn, in
    # z gate negated: sigmoid(z psum) then directly equals 1 - z
    Wc = np.concatenate([w_ih[:H] + w_hh[:H],
                         -(w_ih[H:2 * H] + w_hh[H:2 * H]),
                         w_hh[2 * H:],
                         w_ih[2 * H:]], 0)          # [4H, H]
    bc = np.concatenate([b_ih[:H] + b_hh[:H],
                         -(b_ih[H:2 * H] + b_hh[H:2 * H]),
                         b_hh[2 * H:],
                         b_ih[2 * H:]], 0)          # [4H]

    x0 = np.asarray(embed_table, f32)[0]
    gi0 = w_ih @ x0 + b_ih                          # [3H]
    bc0 = np.concatenate([gi0[:H] + b_hh[:H],
                          -(gi0[H:2 * H] + b_hh[H:2 * H]),
                          b_hh[2 * H:],
                          gi0[2 * H:]], 0)          # [4H]
    W0 = np.concatenate([w_hh[:H], -w_hh[H:2 * H]], 0)  # [2H, H] r,z step-0

    # stationary blocks: wst[kp, ((g*4+q)*4+c)*128 + m] = Wc[g*512+q*128+m,
    #                                                        c*128+kp]
    wst = np.empty((128, 64 * 128), f32)
    for g in range(4):
        for q in range(4):
            for c in range(4):
                blk = ((g * 4 + q) * 4 + c) * 128
                wst[:, blk:blk + 128] = Wc[g * 512 + q * 128:
                                           g * 512 + (q + 1) * 128,
                                           c * 128:(c + 1) * 128].T
    wst0 = np.empty((128, 32 * 128), f32)
    for g in range(2):
        for q in range(4):
            for c in range(4):
                blk = ((g * 4 + q) * 4 + c) * 128
                wst0[:, blk:blk + 128] = W0[g * 512 + q * 128:
                                            g * 512 + (q + 1) * 128,
                                            c * 128:(c + 1) * 128].T

    proj_w = np.asarray(proj_w, f32)                # [V, H]
    pjt = np.empty((128, 4 * VOCAB), f32)
    for c in range(4):
        pjt[:, c * VOCAB:(c + 1) * VOCAB] = proj_w[:, c * 128:(c + 1) * 128].T

    feat = np.asarray(feat, f32)
    common = {
        "wst": wst.astype(f16),
        "wst0": wst0.astype(f16),
        "bt": bc.reshape(1, 2048).astype(f16),
        "bt0": bc0.reshape(1, 2048).astype(f16),
        "ones": np.ones((1, BD), f16),
        "pjt": pjt.astype(f16),
        "pjb": np.asarray(proj_b, f32).reshape(VOCAB, 1),
    }
    maps = []
    for i in range(NCORES):
        fs = feat[i * BD:(i + 1) * BD]              # [BD, H]
        h0g = np.ascontiguousarray(
            fs.T.reshape(4, 128, BD).transpose(1, 0, 2).reshape(128, 128))
        maps.append(dict(common, h0=h0g.astype(f16)))
    return maps


def kernel(feat, embed_table, w_ih, w_hh, b_ih, b_hh, proj_w, proj_b,
           _trace=False):
    nc = _get_nc(STEPS)
    in_maps = _prep_inputs(feat, embed_table, w_ih, w_hh, b_ih, b_hh,
                           proj_w, proj_b)
    res = run_bass_kernel_spmd(nc, in_maps, list(range(NCORES)), trace=_trace)
    out = np.concatenate([res.results[i]["out"] for i in range(NCORES)], 0)
    if _trace:
        kernel.last_exec_time_ns = res.exec_time_ns
        kernel.last_results = res
    return out


# revision 36
# speedup vs baseline: 1.0038x; 1.0038x over previous
"""Trainium2 Bass kernel for nn_CaptionModel (GRU caption decoder).

Model: h0 = feat; x0 = embed[<SOS>]; 200 GRU steps where the output hidden
state is fed back as the next input (x_t = h_t for t >= 1), then a linear
projection of every hidden state to vocab logits, output [B, V, T].

Since x_t == h_t for t >= 1 the two GRU matmuls fuse into one 2048-wide
gate matmul G = h @ Wc.T + bc with Wc = [w_ih_r+w_hh_r; -(w_ih_z+w_hh_z);
w_hh_n; w_ih_n] (z negated so sig(psum) = 1-z), gates r = sig(G0),
z1m = sig(G1) = 1-z, n = tanh(G3 + r*G2), h' = (h - z1m*h) + z1m*n.

KEY STRUCTURE: the GRU recurrence is latency-bound -- the serial loop
[e-operand gate mms -> sigmoid -> a=r*g_hn -> b=a+g_in -> tanh ->
e=z1m*n -> next mms] crosses engines ~6 times at ~200-300ns per handoff
(sem delay + producer pipeline drain), so one recurrence cycles at
~2.5-3us regardless of batch size.  The per-core batch of 32 is split
into TWO independent 16-sample recurrences interleaved on the same
engines (every engine is <60% busy in a single recurrence); each
sub-recurrence then cycles slightly faster (~2.5us, smaller ops) and
they overlap, landing at ~2.6us/step with the PE ~79% busy.

Per half: gate PSUM is 2 single banks (RZ: r|z, NI: hn|in) -- PSUM
dependency tracking is bank-granular, so each bank is fully written
(bias mms first, start=True zeroes the whole 2KB bank) then read.
Burst order per step: bias + u-operand mms (u = z*h is ready early via
the Pool path), then e-operand mms rz-first so the sigmoid fires right
after the 32 e_rz mms instead of after the whole burst.  The sigmoid is
split r-half / z-half (both read the completed RZ bank) so r feeds the
critical a->b->tanh chain one op earlier; z1m feeds the off-loop u-path
(t1 = z1m*h, u = h - t1 on Pool) and the e-mul.  e16 is written in
c-halves so the next burst's e_rz(c01) mms start early; the hist join
(h' = u+e) rides the DVE queue after e16, off the critical loop.

All fp16 rounding sites (t1, u16, e16, n16, hist) are placed exactly as
in the validated single-recurrence kernel: the 200-step feedback loop
amplifies any change in rounding placement, so the math is kept bitwise
identical (rel err 1.92e-2 vs the 2e-2 gate).

Sharding: pure data parallelism, batch 256 -> 32 per core on 8 cores,
weights replicated.
"""

import os
from contextlib import ExitStack

import numpy as np

import concourse.bass as bass
import concourse.tile as tile
from concourse import bacc, mybir
from concourse.bass_utils import run_bass_kernel_spmd

B, H, VOCAB = 256, 512, 100
STEPS = int(os.environ.get("KERNEL_STEPS", "200"))
NCORES = 8
BD = B // NCORES  # 32
HB = BD // 2      # 16 per interleaved half-recurrence
F16 = mybir.dt.float16
F32 = mybir.dt.float32
SIG = mybir.ActivationFunctionType.Sigmoid
TANH = mybir.ActivationFunctionType.Tanh

# gate order in the stationary weight blocks: r z hn in
GATES = ("r", "z", "hn", "in")
GI = {g: i for i, g in enumerate(GATES)}
PROJ_EVERY = 8


def _blk(g, q, c):
    return ((GI[g] * 4 + q) * 4 + c) * 128


def _colof(g, q):
    # within-bank columns: r/hn at q*HB (cols 0:64), z/in at 64+q*HB
    return (64 if g in ("z", "in") else 0) + q * HB


def _build(steps: int):
    nc = bacc.Bacc("TRN2", target_bir_lowering=False, debug=False,
                   num_devices=NCORES)
    T1 = steps + 1

    wst_d = nc.dram_tensor("wst", [128, 64 * 128], F16, kind="ExternalInput").ap()
    wst0_d = nc.dram_tensor("wst0", [128, 32 * 128], F16, kind="ExternalInput").ap()
    h0_d = nc.dram_tensor("h0", [128, 128], F16, kind="ExternalInput").ap()
    bt_d = nc.dram_tensor("bt", [1, 2048], F16, kind="ExternalInput").ap()
    bt0_d = nc.dram_tensor("bt0", [1, 2048], F16, kind="ExternalInput").ap()
    ones_d = nc.dram_tensor("ones", [1, BD], F16, kind="ExternalInput").ap()
    pjt_d = nc.dram_tensor("pjt", [128, 4 * VOCAB], F16, kind="ExternalInput").ap()
    pjb_d = nc.dram_tensor("pjb", [VOCAB, 1], F32, kind="ExternalInput").ap()
    out_d = nc.dram_tensor("out", [BD, VOCAB, steps], F32,
                           kind="ExternalOutput").ap()

    with tile.TileContext(nc) as tc, ExitStack() as ctx:
        sg = ctx.enter_context(tc.tile_pool(name="sg", bufs=1))
        wk = ctx.enter_context(tc.tile_pool(name="wk", bufs=4))

        # small tensors + step-0 operands first so the prologue isn't
        # queued behind the 2MB wst transfer; wst itself is split by gate
        # with hn first (the only wst piece the t=0 prologue needs)
        ones = sg.tile([1, BD], F16)
        nc.sync.dma_start(out=ones, in_=ones_d)
        bt0 = sg.tile([1, 2048], F16)
        nc.sync.dma_start(out=bt0, in_=bt0_d)
        bt = sg.tile([1, 2048], F16)
        nc.sync.dma_start(out=bt, in_=bt_d)
        hist = sg.tile([128, 4, T1, BD], F16, name="hist")
        nc.sync.dma_start(out=hist[:, :, 0, :],
                          in_=h0_d.rearrange("p (q b) -> p q b", q=4))
        pjb = sg.tile([VOCAB, 1], F32)
        nc.sync.dma_start(out=pjb, in_=pjb_d)
        wst0 = sg.tile([128, 32 * 128], F16)
        nc.sync.dma_start(out=wst0, in_=wst0_d)
        wst = sg.tile([128, 64 * 128], F16)
        for g in ("hn", "r", "z", "in"):
            lo, hi = GI[g] * 16 * 128, (GI[g] + 1) * 16 * 128
            nc.sync.dma_start(out=wst[:, lo:hi], in_=wst_d[:, lo:hi])
        pjt = sg.tile([128, 4 * VOCAB], F16)
        nc.sync.dma_start(out=pjt, in_=pjt_d)
        stage = sg.tile([VOCAB, BD * steps], F32, name="stage")

        with tc.tile_pool(name="gps", bufs=1, space="PSUM") as gpool, \
             tc.tile_pool(name="pps", bufs=2, space="PSUM") as ppool:
            # two single banks per half: RZ (r|z) and NI (hn|in).  Bank-
            # granular deps + whole-bank zeroing mean each accumulation
            # group owns a full 2KB bank; 4 gate banks + 2 proj banks = 6.
            # One combined sigmoid over [r|z1m] feeds both the e-chain (r)
            # and the u-path (z1m) in a single Act op.
            RZ = [gpool.tile([128, 512], F32, tag=f"RZ{i}", name=f"RZ{i}")
                  for i in range(2)]
            NI = [gpool.tile([128, 512], F32, tag=f"NI{i}", name=f"NI{i}")
                  for i in range(2)]

            def bank_of(X, g):
                return RZ[X] if g in ("r", "z") else NI[X]

            def bias_u_mms(X, gates, bias_src, w_src, rhs4, wgates=None):
                # first fill phase of each bank: bias mms (first carries
                # start=True, zeroing the bank) then the early-operand
                # weight mms.  wgates limits which gates get weight mms.
                hb0 = X * HB
                if wgates is None:
                    wgates = gates
                firsts = set()
                for g in gates:
                    bank = bank_of(X, g)
                    for q in range(4):
                        col = _colof(g, q)
                        first = id(bank) not in firsts
                        firsts.add(id(bank))
                        nc.tensor.matmul(
                            bank[:, col:col + HB],
                            bias_src[:, GI[g] * 512 + q * 128:
                                     GI[g] * 512 + (q + 1) * 128],
                            ones[:, 0:HB], start=first, stop=False,
                            skip_group_check=True)
                for g in wgates:
                    bank = bank_of(X, g)
                    for q in range(4):
                        for c in range(4):
                            wt = wst0 if (w_src is wst0 and g in ("r", "z")) \
                                else wst
                            nc.tensor.matmul(
                                bank[:, _colof(g, q):_colof(g, q) + HB],
                                wt[:, _blk(g, q, c):_blk(g, q, c) + 128],
                                rhs4[c],
                                start=False,
                                stop=(w_src is wst0 and q == 3 and c == 3),
                                skip_group_check=True)

            def e_mms(X, gates, rhs4, stop_gates, cs=(0, 1, 2, 3)):
                # second fill phase: the late-operand weight mms; gates in
                # stop_gates close their bank's group on their last mm
                for g in gates:
                    bank = bank_of(X, g)
                    for q in range(4):
                        for c in cs:
                            stop = (g in stop_gates and q == 3 and c == cs[-1])
                            nc.tensor.matmul(
                                bank[:, _colof(g, q):_colof(g, q) + HB],
                                wst[:, _blk(g, q, c):_blk(g, q, c) + 128],
                                rhs4[c],
                                start=False, stop=stop,
                                skip_group_check=True)

            def proj_rows(X, r0, nrows):
                hb0 = X * HB
                Pfull = ppool.tile([VOCAB, 512], F32, tag="P", name="Pfull")
                P = Pfull[:, 0:HB * nrows]
                for c in range(4):
                    rhs = hist[:, c, r0:r0 + nrows, hb0:hb0 + HB].rearrange(
                        "p t b -> p b t")
                    nc.tensor.matmul(P, pjt[:, c * VOCAB:(c + 1) * VOCAB], rhs,
                                     start=(c == 0), stop=(c == 3))
                st_sl = stage.rearrange("p (b t) -> p b t", b=BD)[
                    :, hb0:hb0 + HB, r0 - 1:r0 - 1 + nrows]
                nc.vector.tensor_scalar_add(
                    st_sl, P.rearrange("p (b t) -> p b t", b=HB), pjb)

            # ---- step 0: bias0 + w_hh matmuls on h0 (in-region bias-only)
            for X in (0, 1):
                h0c = [hist[:, c, 0, X * HB:X * HB + HB] for c in range(4)]
                bias_u_mms(X, ("r", "z", "hn", "in"), bt0, wst0, h0c,
                           wgates=("r", "z", "hn"))

            next_proj = [1, 1]
            dma_done = 0
            DMA_CHUNK = 25
            for t in range(steps):
                for X in (0, 1):
                    hb0 = X * HB
                    rz_s = wk.tile([128, 128], F32, tag=f"rz{X}")
                    a_s = wk.tile([128, 64], F32, tag=f"a{X}")
                    b_s = wk.tile([128, 64], F32, tag=f"b{X}")
                    n16 = wk.tile([128, 64], F16, tag=f"n{X}")
                    # t1 in f16 keeps the u-path math bitwise identical to the
                    # validated single-recurrence kernel (final max-err of this
                    # 200-step feedback loop is sensitive to rounding placement)
                    t1 = wk.tile([128, 4, HB], F16, tag=f"t1{X}")
                    u16 = wk.tile([128, 4, HB], F16, tag=f"u{X}")
                    e16 = wk.tile([128, 4, HB], F16, tag=f"e{X}")
                    h_t = hist[:, :, t, hb0:hb0 + HB]

                    # chain for step t (banks were filled by iteration t-1).
                    # critical loop: e_rz mms -> sig_r -> a -> b -> tanh ->
                    # e16 -> (next burst's e mms).  The u-path (sig_z1m ->
                    # t1 -> u on Pool) runs in parallel off-loop; splitting
                    # the sigmoid lets the r half feed the chain 54ns
                    # earlier (both halves read the same completed bank)
                    nc.scalar.activation(rz_s[:, 0:64], RZ[X][:, 0:64], SIG)
                    nc.scalar.activation(rz_s[:, 64:128], RZ[X][:, 64:128],
                                         SIG)
                    z1m4 = rz_s[:, 64:128].rearrange("p (q b) -> p q b", q=4)
                    # u path on Pool: t1 = (1-z)*h in f32, u = h - t1
                    nc.gpsimd.tensor_mul(t1, z1m4, h_t)
                    nc.gpsimd.tensor_sub(u16, h_t, t1)
                    # e path on DVE/Act; e16 in c-halves so the next
                    # burst's e_rz(c01) mms start early; the hist join
                    # rides the DVE queue after e16 (off the critical loop)
                    nc.vector.tensor_mul(a_s, rz_s[:, 0:64], NI[X][:, 0:64])
                    nc.vector.tensor_add(b_s, a_s, NI[X][:, 64:128])
                    nc.scalar.activation(n16, b_s, TANH)
                    n4 = n16.rearrange("p (q b) -> p q b", q=4)
                    nc.vector.tensor_mul(e16[:, 0:2, :], z1m4[:, 0:2, :],
                                         n4[:, 0:2, :])
                    nc.vector.tensor_mul(e16[:, 2:4, :], z1m4[:, 2:4, :],
                                         n4[:, 2:4, :])
                    # hist join h(t+1) = u+e on DVE, off the critical loop
                    nc.vector.tensor_add(hist[:, :, t + 1, hb0:hb0 + HB],
                                         u16, e16)

                    # burst for G(t+1): bias + u-operand mms first (u is
                    # ready early via the Pool path), then the e-operand
                    # mms rz-first (c01 then c23) so sig(t+1) fires after
                    # only the 32 e_rz mms
                    if t + 1 < steps:
                        u4 = [u16[:, c, :] for c in range(4)]
                        e4 = [e16[:, c, :] for c in range(4)]
                        bias_u_mms(X, ("r", "z", "hn", "in"), bt, wst, u4)
                        e_mms(X, ("r", "z"), e4, (), cs=(0, 1))
                        e_mms(X, ("r", "z"), e4, ("z",), cs=(2, 3))
                        e_mms(X, ("hn", "in"), e4, ("in",))

                    if next_proj[X] + PROJ_EVERY <= t + 2:
                        proj_rows(X, next_proj[X], PROJ_EVERY)
                        next_proj[X] += PROJ_EVERY

                # stream finalized output chunks once BOTH halves are done
                while dma_done + DMA_CHUNK < min(next_proj):
                    lo = dma_done
                    nc.sync.dma_start(
                        out=out_d[:, :, lo:lo + DMA_CHUNK].rearrange(
                            "b v t -> v b t"),
                        in_=stage.rearrange("p (b t) -> p b t", b=BD)[
                            :, :, lo:lo + DMA_CHUNK])
                    dma_done += DMA_CHUNK

            for X in (0, 1):
                while next_proj[X] <= steps:
                    nrows = min(PROJ_EVERY, steps + 1 - next_proj[X])
                    proj_rows(X, next_proj[X], nrows)
                    next_proj[X] += nrows

        if dma_done < steps:
            nc.sync.dma_start(
                out=out_d[:, :, dma_done:].rearrange("b v t -> v b t"),
                in_=stage.rearrange("p (b t) -> p b t", b=BD)[:, :, dma_done:])
    nc.compile()
    return nc


_CACHE = {}


def _get_nc(steps: int):
    if steps not in _CACHE:
        _CACHE[steps] = _build(steps)
    return _CACHE[steps]


def _prep_inputs(feat, embed_table, w_ih, w_hh, b_ih, b_hh, proj_w, proj_b):
    f32 = np.float32
    f16 = np.float16
    w_ih = np.asarray(w_ih, f32)
    w_hh = np.asarray(w_hh, f32)
    b_ih = np.asarray(b_ih, f32)
    b_hh = np.asarray(b_hh, f32)
    # fused gate weights, gate-major order r, z, hn, in
    # z gate negated: sigmoid(z psum) then directly equals 1 - z
    Wc = np.concatenate([w_ih[:H] + w_hh[:H],
                         -(w_ih[H:2 * H] + w_hh[H:2 * H]),
                         w_hh[2 * H:],
                         w_ih[2 * H:]], 0)          # [4H, H]
    bc = np.concatenate([b_ih[:H] + b_hh[:H],
                         -(b_ih[H:2 * H] + b_hh[H:2 * H]),
                         b_hh[2 * H:],
                         b_ih[2 * H:]], 0)          # [4H]

    x0 = np.asarray(embed_table, f32)[0]
    gi0 = w_ih @ x0 + b_ih                          # [3H]
    bc0 = np.concatenate([gi0[:H] + b_hh[:H],
                          -(gi0[H:2 * H] + b_hh[H:2 * H]),
                          b_hh[2 * H:],
                          gi0[2 * H:]], 0)          # [4H]
    W0 = np.concatenate([w_hh[:H], -w_hh[H:2 * H]], 0)  # [2H, H] r,z step-0

    # stationary blocks: wst[kp, ((g*4+q)*4+c)*128 + m] = Wc[g*512+q*128+m,
    #                                                        c*128+kp]
    wst = np.empty((128, 64 * 128), f32)
    for g in range(4):
        for q in range(4):
            for c in range(4):
                blk = ((g * 4 + q) * 4 + c) * 128
                wst[:, blk:blk + 128] = Wc[g * 512 + q * 128:
                                           g * 512 + (q + 1) * 128,
                                           c * 128:(c + 1) * 128].T
    wst0 = np.empty((128, 32 * 128), f32)
    for g in range(2):
        for q in range(4):
            for c in range(4):
                blk = ((g * 4 + q) * 4 + c) * 128
                wst0[:, blk:blk + 128] = W0[g * 512 + q * 128:
                                            g * 512 + (q + 1) * 128,
                                            c * 128:(c + 1) * 128].T

    proj_w = np.asarray(proj_w, f32)                # [V, H]
    pjt = np.empty((128, 4 * VOCAB), f32)
    for c in range(4):
        pjt[:, c * VOCAB:(c + 1) * VOCAB] = proj_w[:, c * 128:(c + 1) * 128].T

    feat = np.asarray(feat, f32)
    common = {
        "wst": wst.astype(f16),
        "wst0": wst0.astype(f16),
        "bt": bc.reshape(1, 2048).astype(f16),
        "bt0": bc0.reshape(1, 2048).astype(f16),
        "ones": np.ones((1, BD), f16),
        "pjt": pjt.astype(f16),
        "pjb": np.asarray(proj_b, f32).reshape(VOCAB, 1),
    }
    maps = []
    for i in range(NCORES):
        fs = feat[i * BD:(i + 1) * BD]              # [BD, H]
        h0g = np.ascontiguousarray(
            fs.T.reshape(4, 128, BD).transpose(1, 0, 2).reshape(128, 128))
        maps.append(dict(common, h0=h0g.astype(f16)))
    return maps


def kernel(feat, embed_table, w_ih, w_hh, b_ih, b_hh, proj_w, proj_b,
           _trace=False):
    nc = _get_nc(STEPS)
    in_maps = _prep_inputs(feat, embed_table, w_ih, w_hh, b_ih, b_hh,
                           proj_w, proj_b)
    res = run_bass_kernel_spmd(nc, in_maps, list(range(NCORES)), trace=_trace)
    out = np.concatenate([res.results[i]["out"] for i in range(NCORES)], 0)
    if _trace:
        kernel.last_exec_time_ns = res.exec_time_ns
        kernel.last_results = res
    return out


# revision 37
# speedup vs baseline: 1.0161x; 1.0122x over previous
"""Trainium2 Bass kernel for nn_CaptionModel (GRU caption decoder).

Model: h0 = feat; x0 = embed[<SOS>]; 200 GRU steps where the output hidden
state is fed back as the next input (x_t = h_t for t >= 1), then a linear
projection of every hidden state to vocab logits, output [B, V, T].

Since x_t == h_t for t >= 1 the two GRU matmuls fuse into one 2048-wide
gate matmul G = h @ Wc.T + bc with Wc = [w_ih_r+w_hh_r; -(w_ih_z+w_hh_z);
w_hh_n; w_ih_n] (z negated so sig(psum) = 1-z), gates r = sig(G0),
z1m = sig(G1) = 1-z, n = tanh(G3 + r*G2), h' = (h - z1m*h) + z1m*n.

KEY STRUCTURE: the GRU recurrence is latency-bound -- the serial loop
[e-operand gate mms -> sigmoid -> a=r*g_hn -> b=a+g_in -> tanh ->
e=z1m*n -> next mms] crosses engines ~6 times at ~200-300ns per handoff
(sem delay + producer pipeline drain), so one recurrence cycles at
~2.5-3us regardless of batch size.  The per-core batch of 32 is split
into TWO independent 16-sample recurrences interleaved on the same
engines (every engine is <60% busy in a single recurrence); each
sub-recurrence then cycles slightly faster (~2.5us, smaller ops) and
they overlap, landing at ~2.6us/step with the PE ~79% busy.

Per half: gate PSUM is 2 single banks (RZ: r|z, NI: hn|in) -- PSUM
dependency tracking is bank-granular, so each bank is fully written
(bias mms first, start=True zeroes the whole 2KB bank) then read.
Burst order per step: bias + u-operand mms (u = z*h is ready early via
the Pool path), then e-operand mms rz-first so the sigmoid fires right
after the 32 e_rz mms instead of after the whole burst.  The sigmoid is
split r-half / z-half (both read the completed RZ bank) so r feeds the
critical a->b->tanh chain one op earlier; z1m feeds the off-loop u-path
(t1 = z1m*h, u = h - t1 on Pool) and the e-mul.  e16 is written in
c-halves so the next burst's e_rz(c01) mms start early; the hist join
(h' = u+e) rides the DVE queue after e16, off the critical loop.

All fp16 rounding sites (t1, u16, e16, n16, hist) are placed exactly as
in the validated single-recurrence kernel: the 200-step feedback loop
amplifies any change in rounding placement, so the math is kept bitwise
identical (rel err 1.92e-2 vs the 2e-2 gate).

Sharding: pure data parallelism, batch 256 -> 32 per core on 8 cores,
weights replicated.
"""

import os
from contextlib import ExitStack

import numpy as np

import concourse.bass as bass
import concourse.tile as tile
from concourse import bacc, mybir
from concourse.bass_utils import run_bass_kernel_spmd

B, H, VOCAB = 256, 512, 100
STEPS = int(os.environ.get("KERNEL_STEPS", "200"))
NCORES = 8
BD = B // NCORES  # 32
HB = BD // 2      # 16 per interleaved half-recurrence
F16 = mybir.dt.float16
F32 = mybir.dt.float32
SIG = mybir.ActivationFunctionType.Sigmoid
TANH = mybir.ActivationFunctionType.Tanh

# gate order in the stationary weight blocks: r z hn in
GATES = ("r", "z", "hn", "in")
GI = {g: i for i, g in enumerate(GATES)}
PROJ_EVERY = 8


def _blk(g, q, c):
    return ((GI[g] * 4 + q) * 4 + c) * 128


def _colof(g, q):
    # within-bank columns: r/hn at q*HB (cols 0:64), z/in at 64+q*HB
    return (64 if g in ("z", "in") else 0) + q * HB


def _build(steps: int):
    nc = bacc.Bacc("TRN2", target_bir_lowering=False, debug=False,
                   num_devices=NCORES)
    T1 = steps + 1

    wst_d = nc.dram_tensor("wst", [128, 64 * 128], F16, kind="ExternalInput").ap()
    wst0_d = nc.dram_tensor("wst0", [128, 32 * 128], F16, kind="ExternalInput").ap()
    h0_d = nc.dram_tensor("h0", [128, 128], F16, kind="ExternalInput").ap()
    bt_d = nc.dram_tensor("bt", [1, 2048], F16, kind="ExternalInput").ap()
    bt0_d = nc.dram_tensor("bt0", [1, 2048], F16, kind="ExternalInput").ap()
    ones_d = nc.dram_tensor("ones", [1, BD], F16, kind="ExternalInput").ap()
    pjt_d = nc.dram_tensor("pjt", [128, 4 * VOCAB], F16, kind="ExternalInput").ap()
    pjb_d = nc.dram_tensor("pjb", [VOCAB, 1], F32, kind="ExternalInput").ap()
    out_d = nc.dram_tensor("out", [BD, VOCAB, steps], F32,
                           kind="ExternalOutput").ap()

    with tile.TileContext(nc) as tc, ExitStack() as ctx:
        sg = ctx.enter_context(tc.tile_pool(name="sg", bufs=1))
        wk = ctx.enter_context(tc.tile_pool(name="wk", bufs=4))

        # small tensors + step-0 operands first so the prologue isn't
        # queued behind the 2MB wst transfer; wst itself is split by gate
        # with hn first (the only wst piece the t=0 prologue needs)
        ones = sg.tile([1, BD], F16)
        nc.sync.dma_start(out=ones, in_=ones_d)
        bt0 = sg.tile([1, 2048], F16)
        nc.sync.dma_start(out=bt0, in_=bt0_d)
        bt = sg.tile([1, 2048], F16)
        nc.sync.dma_start(out=bt, in_=bt_d)
        hist = sg.tile([128, 4, T1, BD], F16, name="hist")
        nc.sync.dma_start(out=hist[:, :, 0, :],
                          in_=h0_d.rearrange("p (q b) -> p q b", q=4))
        pjb = sg.tile([VOCAB, 1], F32)
        nc.sync.dma_start(out=pjb, in_=pjb_d)
        wst0 = sg.tile([128, 32 * 128], F16)
        nc.sync.dma_start(out=wst0, in_=wst0_d)
        wst = sg.tile([128, 64 * 128], F16)
        for g in ("hn", "r", "z", "in"):
            lo, hi = GI[g] * 16 * 128, (GI[g] + 1) * 16 * 128
            nc.sync.dma_start(out=wst[:, lo:hi], in_=wst_d[:, lo:hi])
        pjt = sg.tile([128, 4 * VOCAB], F16)
        nc.sync.dma_start(out=pjt, in_=pjt_d)
        stage = sg.tile([VOCAB, BD * steps], F32, name="stage")

        with tc.tile_pool(name="gps", bufs=1, space="PSUM") as gpool, \
             tc.tile_pool(name="pps", bufs=2, space="PSUM") as ppool:
            # two single banks per half: RZ (r|z) and NI (hn|in).  Bank-
            # granular deps + whole-bank zeroing mean each accumulation
            # group owns a full 2KB bank; 4 gate banks + 2 proj banks = 6.
            # One combined sigmoid over [r|z1m] feeds both the e-chain (r)
            # and the u-path (z1m) in a single Act op.
            RZ = [gpool.tile([128, 512], F32, tag=f"RZ{i}", name=f"RZ{i}")
                  for i in range(2)]
            NI = [gpool.tile([128, 512], F32, tag=f"NI{i}", name=f"NI{i}")
                  for i in range(2)]

            def bank_of(X, g):
                return RZ[X] if g in ("r", "z") else NI[X]

            def bias_u_mms(X, gates, bias_src, w_src, rhs4, wgates=None):
                # first fill phase of each bank: bias mms (first carries
                # start=True, zeroing the bank) then the early-operand
                # weight mms.  wgates limits which gates get weight mms.
                hb0 = X * HB
                if wgates is None:
                    wgates = gates
                firsts = set()
                for g in gates:
                    bank = bank_of(X, g)
                    for q in range(4):
                        col = _colof(g, q)
                        first = id(bank) not in firsts
                        firsts.add(id(bank))
                        nc.tensor.matmul(
                            bank[:, col:col + HB],
                            bias_src[:, GI[g] * 512 + q * 128:
                                     GI[g] * 512 + (q + 1) * 128],
                            ones[:, 0:HB], start=first, stop=False,
                            skip_group_check=True)
                for g in wgates:
                    bank = bank_of(X, g)
                    for q in range(4):
                        for c in range(4):
                            wt = wst0 if (w_src is wst0 and g in ("r", "z")) \
                                else wst
                            nc.tensor.matmul(
                                bank[:, _colof(g, q):_colof(g, q) + HB],
                                wt[:, _blk(g, q, c):_blk(g, q, c) + 128],
                                rhs4[c],
                                start=False,
                                stop=(w_src is wst0 and q == 3 and c == 3),
                                skip_group_check=True)

            def e_mms(X, gates, rhs4, stop_gates, cs=(0, 1, 2, 3)):
                # second fill phase: the late-operand weight mms; gates in
                # stop_gates close their bank's group on their last mm
                for g in gates:
                    bank = bank_of(X, g)
                    for q in range(4):
                        for c in cs:
                            stop = (g in stop_gates and q == 3 and c == cs[-1])
                            nc.tensor.matmul(
                                bank[:, _colof(g, q):_colof(g, q) + HB],
                                wst[:, _blk(g, q, c):_blk(g, q, c) + 128],
                                rhs4[c],
                                start=False, stop=stop,
                                skip_group_check=True)

            proj_P = [None, None]

            def proj_chunk(X, r0, nrows, c):
                # one c-chunk of the projection; spreading the 4 chunks
                # over 4 iterations keeps the PE load per step smooth
                # (the P psum group stays open across iterations)
                hb0 = X * HB
                if c == 0:
                    proj_P[X] = ppool.tile([VOCAB, 512], F32, tag="P",
                                           name="Pfull")
                P = proj_P[X][:, 0:HB * nrows]
                rhs = hist[:, c, r0:r0 + nrows, hb0:hb0 + HB].rearrange(
                    "p t b -> p b t")
                nc.tensor.matmul(P, pjt[:, c * VOCAB:(c + 1) * VOCAB], rhs,
                                 start=(c == 0), stop=(c == 3))
                if c == 3:
                    st_sl = stage.rearrange("p (b t) -> p b t", b=BD)[
                        :, hb0:hb0 + HB, r0 - 1:r0 - 1 + nrows]
                    nc.vector.tensor_scalar_add(
                        st_sl, P.rearrange("p (b t) -> p b t", b=HB), pjb)

            def proj_rows(X, r0, nrows):
                for c in range(4):
                    proj_chunk(X, r0, nrows, c)

            # ---- step 0: bias0 + w_hh matmuls on h0 (in-region bias-only)
            for X in (0, 1):
                h0c = [hist[:, c, 0, X * HB:X * HB + HB] for c in range(4)]
                bias_u_mms(X, ("r", "z", "hn", "in"), bt0, wst0, h0c,
                           wgates=("r", "z", "hn"))

            next_proj = [1, 1]
            proj_c = [0, 0]
            dma_done = 0
            DMA_CHUNK = 25
            for t in range(steps):
                for X in (0, 1):
                    hb0 = X * HB
                    rz_s = wk.tile([128, 128], F32, tag=f"rz{X}")
                    a_s = wk.tile([128, 64], F32, tag=f"a{X}")
                    b_s = wk.tile([128, 64], F32, tag=f"b{X}")
                    n16 = wk.tile([128, 64], F16, tag=f"n{X}")
                    # t1 in f16 keeps the u-path math bitwise identical to the
                    # validated single-recurrence kernel (final max-err of this
                    # 200-step feedback loop is sensitive to rounding placement)
                    t1 = wk.tile([128, 4, HB], F16, tag=f"t1{X}")
                    u16 = wk.tile([128, 4, HB], F16, tag=f"u{X}")
                    e16 = wk.tile([128, 4, HB], F16, tag=f"e{X}")
                    h_t = hist[:, :, t, hb0:hb0 + HB]

                    # chain for step t (banks were filled by iteration t-1).
                    # critical loop: e_rz mms -> sig_r -> a -> b -> tanh ->
                    # e16 -> (next burst's e mms).  The u-path (sig_z1m ->
                    # t1 -> u on Pool) runs in parallel off-loop; splitting
                    # the sigmoid lets the r half feed the chain 54ns
                    # earlier (both halves read the same completed bank)
                    nc.scalar.activation(rz_s[:, 0:64], RZ[X][:, 0:64], SIG)
                    nc.scalar.activation(rz_s[:, 64:128], RZ[X][:, 64:128],
                                         SIG)
                    z1m4 = rz_s[:, 64:128].rearrange("p (q b) -> p q b", q=4)
                    # u path on Pool: t1 = (1-z)*h in f32, u = h - t1
                    nc.gpsimd.tensor_mul(t1, z1m4, h_t)
                    nc.gpsimd.tensor_sub(u16, h_t, t1)
                    # e path on DVE/Act; e16 in c-halves so the next
                    # burst's e_rz(c01) mms start early; the hist join
                    # rides the DVE queue after e16 (off the critical loop)
                    nc.vector.tensor_mul(a_s, rz_s[:, 0:64], NI[X][:, 0:64])
                    nc.vector.tensor_add(b_s, a_s, NI[X][:, 64:128])
                    nc.scalar.activation(n16, b_s, TANH)
                    n4 = n16.rearrange("p (q b) -> p q b", q=4)
                    nc.vector.tensor_mul(e16[:, 0:2, :], z1m4[:, 0:2, :],
                                         n4[:, 0:2, :])
                    nc.vector.tensor_mul(e16[:, 2:4, :], z1m4[:, 2:4, :],
                                         n4[:, 2:4, :])
                    # hist join h(t+1) = u+e on DVE, off the critical loop
                    nc.vector.tensor_add(hist[:, :, t + 1, hb0:hb0 + HB],
                                         u16, e16)

                    # burst for G(t+1): bias + u-operand mms first (u is
                    # ready early via the Pool path), then the e-operand
                    # mms rz-first (c01 then c23) so sig(t+1) fires after
                    # only the 32 e_rz mms
                    if t + 1 < steps:
                        u4 = [u16[:, c, :] for c in range(4)]
                        e4 = [e16[:, c, :] for c in range(4)]
                        bias_u_mms(X, ("r", "z", "hn", "in"), bt, wst, u4)
                        e_mms(X, ("r", "z"), e4, (), cs=(0, 1))
                        e_mms(X, ("r", "z"), e4, ("z",), cs=(2, 3))
                        e_mms(X, ("hn", "in"), e4, ("in",))

                    if proj_c[X] > 0 or next_proj[X] + PROJ_EVERY <= t + 2:
                        proj_chunk(X, next_proj[X], PROJ_EVERY, proj_c[X])
                        if proj_c[X] == 3:
                            proj_c[X] = 0
                            next_proj[X] += PROJ_EVERY
                        else:
                            proj_c[X] += 1

                # stream finalized output chunks once BOTH halves are done
                while dma_done + DMA_CHUNK < min(next_proj):
                    lo = dma_done
                    nc.sync.dma_start(
                        out=out_d[:, :, lo:lo + DMA_CHUNK].rearrange(
                            "b v t -> v b t"),
                        in_=stage.rearrange("p (b t) -> p b t", b=BD)[
                            :, :, lo:lo + DMA_CHUNK])
                    dma_done += DMA_CHUNK

            for X in (0, 1):
                while proj_c[X] > 0:
                    proj_chunk(X, next_proj[X], PROJ_EVERY, proj_c[X])
                    if proj_c[X] == 3:
                        proj_c[X] = 0
                        next_proj[X] += PROJ_EVERY
                    else:
                        proj_c[X] += 1
                while next_proj[X] <= steps:
                    nrows = min(PROJ_EVERY, steps + 1 - next_proj[X])
                    proj_rows(X, next_proj[X], nrows)
                    next_proj[X] += nrows

        if dma_done < steps:
            nc.sync.dma_start(
                out=out_d[:, :, dma_done:].rearrange("b v t -> v b t"),
                in_=stage.rearrange("p (b t) -> p b t", b=BD)[:, :, dma_done:])
    nc.compile()
    return nc


_CACHE = {}


def _get_nc(steps: int):
    if steps not in _CACHE:
        _CACHE[steps] = _build(steps)
    return _CACHE[steps]


def _prep_inputs(feat, embed_table, w_ih, w_hh, b_ih, b_hh, proj_w, proj_b):
    f32 = np.float32
    f16 = np.float16
    w_ih = np.asarray(w_ih, f32)
    w_hh = np.asarray(w_hh, f32)
    b_ih = np.asarray(b_ih, f32)
    b_hh = np.asarray(b_hh, f32)
    # fused gate weights, gate-major order r, z, hn, in
    # z gate negated: sigmoid(z psum) then directly equals 1 - z
    Wc = np.concatenate([w_ih[:H] + w_hh[:H],
                         -(w_ih[H:2 * H] + w_hh[H:2 * H]),
                         w_hh[2 * H:],
                         w_ih[2 * H:]], 0)          # [4H, H]
    bc = np.concatenate([b_ih[:H] + b_hh[:H],
                         -(b_ih[H:2 * H] + b_hh[H:2 * H]),
                         b_hh[2 * H:],
                         b_ih[2 * H:]], 0)          # [4H]

    x0 = np.asarray(embed_table, f32)[0]
    gi0 = w_ih @ x0 + b_ih                          # [3H]
    bc0 = np.concatenate([gi0[:H] + b_hh[:H],
                          -(gi0[H:2 * H] + b_hh[H:2 * H]),
                          b_hh[2 * H:],
                          gi0[2 * H:]], 0)          # [4H]
    W0 = np.concatenate([w_hh[:H], -w_hh[H:2 * H]], 0)  # [2H, H] r,z step-0

    # stationary blocks: wst[kp, ((g*4+q)*4+c)*128 + m] = Wc[g*512+q*128+m,
    #                                                        c*128+kp]
    wst = np.empty((128, 64 * 128), f32)
    for g in range(4):
        for q in range(4):
            for c in range(4):
                blk = ((g * 4 + q) * 4 + c) * 128
                wst[:, blk:blk + 128] = Wc[g * 512 + q * 128:
                                           g * 512 + (q + 1) * 128,
                                           c * 128:(c + 1) * 128].T
    wst0 = np.empty((128, 32 * 128), f32)
    for g in range(2):
        for q in range(4):
            for c in range(4):
                blk = ((g * 4 + q) * 4 + c) * 128
                wst0[:, blk:blk + 128] = W0[g * 512 + q * 128:
                                            g * 512 + (q + 1) * 128,
                                            c * 128:(c + 1) * 128].T

    proj_w = np.asarray(proj_w, f32)                # [V, H]
    pjt = np.empty((128, 4 * VOCAB), f32)
    for c in range(4):
        pjt[:, c * VOCAB:(c + 1) * VOCAB] = proj_w[:, c * 128:(c + 1) * 128].T

    feat = np.asarray(feat, f32)
    common = {
        "wst": wst.astype(f16),
        "wst0": wst0.astype(f16),
        "bt": bc.reshape(1, 2048).astype(f16),
        "bt0": bc0.reshape(1, 2048).astype(f16),
        "ones": np.ones((1, BD), f16),
        "pjt": pjt.astype(f16),
        "pjb": np.asarray(proj_b, f32).reshape(VOCAB, 1),
    }
    maps = []
    for i in range(NCORES):
        fs = feat[i * BD:(i + 1) * BD]              # [BD, H]
        h0g = np.ascontiguousarray(
            fs.T.reshape(4, 128, BD).transpose(1, 0, 2).reshape(128, 128))
        maps.append(dict(common, h0=h0g.astype(f16)))
    return maps


def kernel(feat, embed_table, w_ih, w_hh, b_ih, b_hh, proj_w, proj_b,
           _trace=False):
    nc = _get_nc(STEPS)
    in_maps = _prep_inputs(feat, embed_table, w_ih, w_hh, b_ih, b_hh,
                           proj_w, proj_b)
    res = run_bass_kernel_spmd(nc, in_maps, list(range(NCORES)), trace=_trace)
    out = np.concatenate([res.results[i]["out"] for i in range(NCORES)], 0)
    if _trace:
        kernel.last_exec_time_ns = res.exec_time_ns
        kernel.last_results = res
    return out
